# revision 10
# baseline (speedup 1.0000x reference)
"""GQA multi-head attention (B=2, S=2048, H=2048, 16 Q heads / 4 KV heads, RoPE,
causal) on 8 Trainium2 NeuronCores.

Sharding: tensor-parallel over GQA groups (4 groups, each 4 Q heads + 1 KV head)
x data-parallel over batch (2). Core c handles batch b = c // 4, group g = c % 4.
Column-parallel q/k/v projections, row-parallel o_proj; the 4 partial o_proj
outputs per batch are summed on the host.

Per-core kernel (all matmuls bf16 with fp32 PSUM accumulation):
  phase 1: Q^T/K^T/V^T projections from X^T, RoPE on Q/K (half-swap done with a
           PSUM->SBUF DMA partition swap), V transposed to natural layout on PE.
  phase 2: flash-style attention in S^T (keys x queries) layout: S^T = K^T.T Q^T
           per 128-key block, exp on ScalarE (no max subtraction - scores are
           bounded), causal mask via a 128x128 0/1 multiply on the diagonal
           blocks, key-axis row sums via a ones-vector matmul, O^T = V^T P^T
           accumulated in PSUM, normalized by 1/rowsum at the end.
  phase 3: row-parallel o_proj Y = O^T.T @ Wo_g^T accumulated over the 4 local
           heads, DMA'd straight from PSUM to HBM.
"""

import sys

for _p in ("/root/.axon_site", "/root/.axon_site/_ro/trn_rl_repo",
           "/root/.axon_site/_ro/pypackages", "/opt/trn_rl_repo"):
    if _p not in sys.path:
        sys.path.append(_p)

import numpy as np
import ml_dtypes

import concourse.bass as bass
import concourse.tile as tile
import concourse.mybir as mybir
from concourse import bacc
from concourse.bass import ts
from concourse.bass_utils import run_bass_kernel_spmd
from concourse.masks import make_identity, make_upper_triangular
from contextlib import ExitStack

BF16 = ml_dtypes.bfloat16
P = 128
S = 2048
H = 2048
NH = 4          # Q heads per core
DQ = NH * P     # 512
NCH = H // P    # 16 hidden chunks
NKB = S // P    # 16 key blocks
QTS = 512       # query tile (phase 2)
SCALE = 1.0 / float(np.sqrt(128.0))


def build_nc():
    f32 = mybir.dt.float32
    bf16 = mybir.dt.bfloat16
    nc = bacc.Bacc("TRN2", target_bir_lowering=False, debug=False)

    xT = nc.dram_tensor("xT", (H, S), bf16, kind="ExternalInput").ap()
    wqT = nc.dram_tensor("wqT", (H, DQ), bf16, kind="ExternalInput").ap()
    wkT = nc.dram_tensor("wkT", (H, P), bf16, kind="ExternalInput").ap()
    wvT = nc.dram_tensor("wvT", (H, P), bf16, kind="ExternalInput").ap()
    woT = nc.dram_tensor("woT", (DQ, H), bf16, kind="ExternalInput").ap()
    cosT = nc.dram_tensor("cosT", (P, S), f32, kind="ExternalInput").ap()
    srT = nc.dram_tensor("sinrotT", (P, S), f32, kind="ExternalInput").ap()
    y = nc.dram_tensor("y", (S, H), bf16, kind="ExternalOutput").ap()
    rss2 = nc.dram_tensor("rss2", (NH * S // QTS, QTS), f32).ap()  # recip scratch

    Exp = mybir.ActivationFunctionType.Exp

    with ExitStack() as ctx:
        tc = ctx.enter_context(tile.TileContext(nc))
        singles = ctx.enter_context(tc.tile_pool(name="singles", bufs=1))

        # Batched input loads: one strided DMA per group-of-chunks (per-DMA
        # issue on the Sync queue costs ~0.6us; 54 small loads would delay the
        # first matmul by ~30us). K/V weights first so the chunk-streamed K/V
        # projections start immediately.
        xT_sb = singles.tile([P, NCH, S], bf16)
        wqT_sb = singles.tile([P, NCH, DQ], bf16)
        wkT_sb = singles.tile([P, NCH, P], bf16)
        wvT_sb = singles.tile([P, NCH, P], bf16)
        xTr = xT.rearrange("(c p) s -> p c s", p=P)
        wqTr = wqT.rearrange("(c p) m -> p c m", p=P)
        cos_sb = singles.tile([P, S], f32)
        sr_sb = singles.tile([P, S], f32)
        nc.sync.dma_start(wkT_sb[:, :, :], wkT.rearrange("(c p) m -> p c m", p=P))
        for c in range(2):  # first chunks individually: K proj streams earliest
            nc.sync.dma_start(xT_sb[:, c, :], xTr[:, c, :])
        nc.sync.dma_start(cos_sb, cosT)
        nc.sync.dma_start(sr_sb, srT)
        nc.sync.dma_start(xT_sb[:, 2:4, :], xTr[:, 2:4, :])
        nc.sync.dma_start(wvT_sb[:, :, :], wvT.rearrange("(c p) m -> p c m", p=P))
        nc.sync.dma_start(xT_sb[:, 4:8, :], xTr[:, 4:8, :])
        nc.sync.dma_start(wqT_sb[:, 0:8, :], wqTr[:, 0:8, :])
        nc.sync.dma_start(xT_sb[:, 8:12, :], xTr[:, 8:12, :])
        nc.sync.dma_start(wqT_sb[:, 8:16, :], wqTr[:, 8:16, :])
        nc.sync.dma_start(xT_sb[:, 12:16, :], xTr[:, 12:16, :])
        woT_sb = singles.tile([P, NH, H], bf16)
        nc.sync.dma_start(woT_sb[:, :, :], woT.rearrange("(c p) m -> p c m", p=P))

        # Additive causal mask, applied on the PE: a matmul of masknegT.T @ I
        # accumulated into the scores PSUM adds -1e9 where key > query.
        masknegT = singles.tile([P, P], bf16)
        make_upper_triangular(nc, masknegT[:], val=-1e9, diag=False)
        ident = singles.tile([P, P], bf16)
        make_identity(nc, ident[:])
        ones = singles.tile([P, 1], bf16)
        nc.vector.memset(ones[:], 1.0)

        QT_sb = singles.tile([P, NH, S], bf16)
        KT_sb = singles.tile([P, S], bf16)
        VT_sb = singles.tile([P, S], bf16)
        Vn_sb = singles.tile([P, NKB, P], bf16)
        OT_sb = singles.tile([P, NH, S], bf16)

        # ---------------- phase 1: projections + RoPE + V transpose ---------
        with tc.tile_pool(name="pj", bufs=4, space="PSUM") as pj, \
             tc.tile_pool(name="rope", bufs=2) as rp, \
             tc.tile_pool(name="vtp", bufs=2, space="PSUM") as vtp:

            # PE warmup during the initial DMA window: ~100 no-dep matmuls keep
            # the HAM activity monitor busy so real matmuls start at 2.4 GHz.
            warm = vtp.tile([P, P], f32, tag="warm", bufs=1)
            for _ in range(48):
                nc.tensor.matmul(warm[:, :], ident[:], ident[:],
                                 start=True, stop=True)

            def proj(w_sb, head, tok):
                ps = pj.tile([P, QTS], f32, tag="proj")
                for c in range(NCH):
                    nc.tensor.matmul(
                        ps[:, :], w_sb[:, c, ts(head, P)], xT_sb[:, c, ts(tok, QTS)],
                        start=(c == 0), stop=(c == NCH - 1))
                return ps

            def proj_streamed(w_sb):
                # chunk-outer: all 4 token tiles accumulate as xT chunks land,
                # so the PE starts ~1 chunk after the first DMA instead of
                # waiting for the full xT load.
                pss = [pj.tile([P, QTS], f32, tag="proj", name=f"pstr{t}")
                       for t in range(4)]
                for c in range(NCH):
                    for t in range(4):
                        nc.tensor.matmul(
                            pss[t][:, :], w_sb[:, c, :], xT_sb[:, c, ts(t, QTS)],
                            start=(c == 0), stop=(c == NCH - 1))
                return pss

            def rope(ps, out_region, tok):
                qf = rp.tile([P, QTS], f32, tag="qf")
                nc.scalar.copy(qf[:, :], ps[:, :])
                sw = rp.tile([P, QTS], f32, tag="swap")
                nc.sync.dma_start(sw[0:64, :], qf[64:128, :])
                nc.sync.dma_start(sw[64:128, :], qf[0:64, :])
                t1 = rp.tile([P, QTS], f32, tag="t1")
                nc.vector.tensor_mul(t1[:, :], ps[:, :], cos_sb[:, ts(tok, QTS)])
                t2 = rp.tile([P, QTS], f32, tag="t2")
                nc.vector.tensor_mul(t2[:, :], sw[:, :], sr_sb[:, ts(tok, QTS)])
                nc.vector.tensor_add(out_region, t1[:, :], t2[:, :])

            # K and V first (chunk-streamed): phase 2 consumes them first.
            for t, ps in enumerate(proj_streamed(wkT_sb)):
                rope(ps, KT_sb[:, ts(t, QTS)], t)
            for t, ps in enumerate(proj_streamed(wvT_sb)):
                nc.vector.tensor_copy(VT_sb[:, ts(t, QTS)], ps[:, :])
                for b in range(4 * t, 4 * t + 4):
                    tp = vtp.tile([P, P], bf16, tag="vt")
                    nc.tensor.transpose(tp[:, :], VT_sb[:, ts(b, P)], ident[:])
                    nc.vector.tensor_copy(Vn_sb[:, b, :], tp[:, :])
            # t-outer so attention column t=0 (all heads) unblocks first
            for t in range(S // QTS):
                for h in range(NH):
                    ps = proj(wqT_sb, h, t)
                    rope(ps, QT_sb[:, h, ts(t, QTS)], t)

        # ------- phase 2 + 3: attention with o_proj matmuls injected as PE
        # filler between attention blocks.  The PE queue is strict FIFO, so
        # while an rs/osum matmul waits on its exp, only instructions already
        # ahead of it can run: we pop 1-2 ready o_proj matmuls (from the
        # previous, completed q column) after each scores matmul so the PE
        # always has ~4 issued-and-ready matmuls per exp in flight.
        from collections import deque
        fillers = deque()

        def pop_fillers(n):
            k = 0
            while k < n and fillers:
                k += fillers.popleft()()

        with tc.tile_pool(name="spp", bufs=2, space="PSUM") as spp, \
             tc.tile_pool(name="opp", bufs=2, space="PSUM") as opp, \
             tc.tile_pool(name="rsp", bufs=2, space="PSUM") as rsp, \
             tc.tile_pool(name="ypp", bufs=2, space="PSUM") as ypp, \
             tc.tile_pool(name="ptp", bufs=4) as ptp, \
             tc.tile_pool(name="yop", bufs=4) as yop, \
             tc.tile_pool(name="nrm", bufs=3) as nrm:

            def make_oproj_fillers(t):
                # 16 groups of 4 accumulating matmuls + a copy/DMA trailer.
                # Group h-order means the h=3 matmul (which needs the last
                # norm-mul of column t) pops a few blocks into column t+1.
                for tb in range(4 * t, 4 * t + 4):
                    for ho in range(H // QTS):
                        yp = ypp.tile([P, QTS], f32, tag="yp",
                                      name=f"yp{tb}_{ho}")

                        def mm(h, yp=yp, tb=tb, ho=ho):
                            nc.tensor.matmul(yp[:, :], OT_sb[:, h, ts(tb, P)],
                                             woT_sb[:, h, ts(ho, QTS)],
                                             start=(h == 0), stop=(h == NH - 1))
                            return 1

                        def trailer(yp=yp, tb=tb, ho=ho):
                            yo = yop.tile([P, QTS], bf16, tag="yo")
                            nc.vector.tensor_copy(yo[:, :], yp[:, :])
                            nc.sync.dma_start(y[ts(tb, P), ts(ho, QTS)], yo[:, :])
                            return 0

                        for h in range(NH):
                            fillers.append(lambda h=h, mm=mm: mm(h))
                        fillers.append(trailer)

            for t in range(S // QTS):
                for h in range(NH):
                    qs = QTS * t
                    nj = 4 * t + 4          # key blocks for this q tile
                    osum = opp.tile([P, QTS], f32, tag="osum")
                    rs = rsp.tile([1, QTS], f32, tag="rs")
                    for j in range(nj):
                        co = max(0, P * j - qs)
                        diag = j >= 4 * t
                        sp = spp.tile([P, QTS], f32, tag="sp")
                        nc.tensor.matmul(
                            sp[:, co:QTS], KT_sb[:, ts(j, P)],
                            QT_sb[:, h, qs + co:qs + QTS],
                            start=True, stop=not diag)
                        if diag:
                            nc.tensor.matmul(sp[:, co:co + P], masknegT[:],
                                             ident[:], start=False, stop=True)
                        pop_fillers(2)
                        pt = ptp.tile([P, QTS], bf16, tag="pt")
                        nc.scalar.activation(pt[:, co:QTS], sp[:, co:QTS], Exp,
                                             scale=SCALE)
                        nc.tensor.matmul(rs[0:1, co:QTS], ones[:], pt[:, co:QTS],
                                         start=(j == 0), stop=(j == nj - 1))
                        nc.tensor.matmul(osum[:, co:QTS], Vn_sb[:, j, :],
                                         pt[:, co:QTS],
                                         start=(j == 0), stop=(j == nj - 1))
                    # 1/rowsum via 2-op Newton-Raphson approx (~2 ULP) straight
                    # off PSUM, then gpsimd broadcast across partitions.
                    rsc = nrm.tile([1, QTS], f32, tag="rsc")
                    rrow = nrm.tile([1, QTS], f32, tag="rrow")
                    nc.vector.reciprocal_approx_accurate(rrow[:, :], rs[0:1, :],
                                                         rsc[:, :])
                    recipB = nrm.tile([P, QTS], f32, tag="recipB")
                    nc.gpsimd.partition_broadcast(recipB[:, :], rrow[:, :])
                    nc.vector.tensor_mul(OT_sb[:, h, qs:qs + QTS], osum[:, :],
                                         recipB[:, :])
                make_oproj_fillers(t)
            while fillers:
                fillers.popleft()()

    nc.compile()
    return nc


_NC_CACHE = None


def _get_nc():
    global _NC_CACHE
    if _NC_CACHE is None:
        _NC_CACHE = build_nc()
    return _NC_CACHE


def make_in_maps(hidden_states, position_ids, wq, wk, wv, wo):
    """Host-side sharding: 8 cores = (batch b = core//4) x (GQA group g = core%4)."""
    in_maps = []
    xTs, coss, srs = {}, {}, {}
    for b in range(2):
        xTs[b] = np.ascontiguousarray(hidden_states[b].T).astype(BF16)
        inv = 1.0 / (10000.0 ** (np.arange(0, P, 2, dtype=np.float64) / P))
        invd = np.concatenate([inv, inv]).astype(np.float64)
        fr = invd[:, None] * position_ids[b].astype(np.float64)[None, :]
        coss[b] = np.cos(fr).astype(np.float32)
        sr = np.sin(fr).astype(np.float32)
        sr[:64] *= -1.0
        srs[b] = sr
    shards = {}
    for g in range(4):
        shards[g] = dict(
            wqT=np.ascontiguousarray(wq[DQ * g:DQ * (g + 1)].T).astype(BF16),
            wkT=np.ascontiguousarray(wk[P * g:P * (g + 1)].T).astype(BF16),
            wvT=np.ascontiguousarray(wv[P * g:P * (g + 1)].T).astype(BF16),
            woT=np.ascontiguousarray(wo[:, DQ * g:DQ * (g + 1)].T).astype(BF16),
        )
    for core in range(8):
        b, g = core // 4, core % 4
        in_maps.append(dict(xT=xTs[b], cosT=coss[b], sinrotT=srs[b], **shards[g]))
    return in_maps


def kernel(hidden_states, position_ids, wq, wk, wv, wo, **run_kwargs):
    nc = _get_nc()
    in_maps = make_in_maps(np.asarray(hidden_states), np.asarray(position_ids),
                           np.asarray(wq), np.asarray(wk), np.asarray(wv),
                           np.asarray(wo))
    res = run_bass_kernel_spmd(nc, in_maps, core_ids=list(range(8)), **run_kwargs)
    out = np.zeros((2, S, H), np.float32)
    for core in range(8):
        out[core // 4] += res.results[core]["y"].astype(np.float32)
    if run_kwargs:
        kernel.last_results = res
    return out



# revision 13
# speedup vs baseline: 1.1266x; 1.1266x over previous
"""GQA multi-head attention (B=2, S=2048, H=2048, 16 Q heads / 4 KV heads, RoPE,
causal) on 8 Trainium2 NeuronCores.

Sharding: tensor-parallel over GQA groups (4 groups, each 4 Q heads + 1 KV head)
x data-parallel over batch (2). Core c handles batch b = c // 4, group g = c % 4.
Column-parallel q/k/v projections, row-parallel o_proj; the 4 partial o_proj
outputs per batch are summed on the host.

Per-core kernel (all matmuls bf16 with fp32 PSUM accumulation):
  phase 1: Q^T/K^T/V^T projections from X^T, RoPE on Q/K (half-swap done with a
           PSUM->SBUF DMA partition swap), V transposed to natural layout on PE.
  phase 2: flash-style attention in S^T (keys x queries) layout: S^T = K^T.T Q^T
           per 128-key block, exp on ScalarE (no max subtraction - scores are
           bounded), causal mask via a 128x128 0/1 multiply on the diagonal
           blocks, key-axis row sums via a ones-vector matmul, O^T = V^T P^T
           accumulated in PSUM, normalized by 1/rowsum at the end.
  phase 3: row-parallel o_proj Y = O^T.T @ Wo_g^T accumulated over the 4 local
           heads, DMA'd straight from PSUM to HBM.
"""

import sys

for _p in ("/root/.axon_site", "/root/.axon_site/_ro/trn_rl_repo",
           "/root/.axon_site/_ro/pypackages", "/opt/trn_rl_repo"):
    if _p not in sys.path:
        sys.path.append(_p)

import numpy as np
import ml_dtypes

import concourse.bass as bass
import concourse.tile as tile
import concourse.mybir as mybir
from concourse import bacc
from concourse.bass import ts
from concourse.bass_utils import run_bass_kernel_spmd
from concourse.masks import make_identity, make_upper_triangular
from contextlib import ExitStack

BF16 = ml_dtypes.bfloat16
P = 128
S = 2048
H = 2048
NH = 4          # Q heads per core
DQ = NH * P     # 512
NCH = H // P    # 16 hidden chunks
NKB = S // P    # 16 key blocks
QTS = 512       # query tile (phase 2)
SCALE = 1.0 / float(np.sqrt(128.0))


def build_nc():
    f32 = mybir.dt.float32
    bf16 = mybir.dt.bfloat16
    nc = bacc.Bacc("TRN2", target_bir_lowering=False, debug=False)

    xT = nc.dram_tensor("xT", (H, S), bf16, kind="ExternalInput").ap()
    wqT = nc.dram_tensor("wqT", (H, DQ), bf16, kind="ExternalInput").ap()
    wkT = nc.dram_tensor("wkT", (H, P), bf16, kind="ExternalInput").ap()
    wvT = nc.dram_tensor("wvT", (H, P), bf16, kind="ExternalInput").ap()
    woT = nc.dram_tensor("woT", (DQ, H), bf16, kind="ExternalInput").ap()
    cosT = nc.dram_tensor("cosT", (P, S), f32, kind="ExternalInput").ap()
    srT = nc.dram_tensor("sinrotT", (P, S), f32, kind="ExternalInput").ap()
    y = nc.dram_tensor("y", (S, H), bf16, kind="ExternalOutput").ap()
    rss2 = nc.dram_tensor("rss2", (NH * S // QTS, QTS), f32).ap()  # recip scratch

    Exp = mybir.ActivationFunctionType.Exp

    with ExitStack() as ctx:
        tc = ctx.enter_context(tile.TileContext(nc))
        singles = ctx.enter_context(tc.tile_pool(name="singles", bufs=1))

        # Batched input loads: one strided DMA per group-of-chunks (per-DMA
        # issue on the Sync queue costs ~0.6us; 54 small loads would delay the
        # first matmul by ~30us). K/V weights first so the chunk-streamed K/V
        # projections start immediately.
        xT_sb = singles.tile([P, NCH, S], bf16)
        wqT_sb = singles.tile([P, NCH, DQ], bf16)
        wkT_sb = singles.tile([P, NCH, P], bf16)
        wvT_sb = singles.tile([P, NCH, P], bf16)
        xTr = xT.rearrange("(c p) s -> p c s", p=P)
        wqTr = wqT.rearrange("(c p) m -> p c m", p=P)
        cos_sb = singles.tile([P, S], f32)
        sr_sb = singles.tile([P, S], f32)
        nc.sync.dma_start(wkT_sb[:, :, :], wkT.rearrange("(c p) m -> p c m", p=P))
        for c in range(2):  # first chunks individually: K proj streams earliest
            nc.sync.dma_start(xT_sb[:, c, :], xTr[:, c, :])
        nc.sync.dma_start(cos_sb, cosT)
        nc.sync.dma_start(sr_sb, srT)
        nc.sync.dma_start(xT_sb[:, 2:4, :], xTr[:, 2:4, :])
        nc.sync.dma_start(wvT_sb[:, :, :], wvT.rearrange("(c p) m -> p c m", p=P))
        nc.sync.dma_start(xT_sb[:, 4:8, :], xTr[:, 4:8, :])
        nc.sync.dma_start(wqT_sb[:, 0:8, :], wqTr[:, 0:8, :])
        nc.sync.dma_start(xT_sb[:, 8:12, :], xTr[:, 8:12, :])
        nc.sync.dma_start(wqT_sb[:, 8:16, :], wqTr[:, 8:16, :])
        nc.sync.dma_start(xT_sb[:, 12:16, :], xTr[:, 12:16, :])
        woT_sb = singles.tile([P, NH, H], bf16)
        nc.sync.dma_start(woT_sb[:, :, :], woT.rearrange("(c p) m -> p c m", p=P))

        # Additive causal mask, applied on the PE: a matmul of masknegT.T @ I
        # accumulated into the scores PSUM adds -1e9 where key > query.
        masknegT = singles.tile([P, P], bf16)
        make_upper_triangular(nc, masknegT[:], val=-1e9, diag=False)
        ident = singles.tile([P, P], bf16)
        make_identity(nc, ident[:])
        ones = singles.tile([P, 1], bf16)
        nc.vector.memset(ones[:], 1.0)

        QT_sb = singles.tile([P, NH, S], bf16)
        KT_sb = singles.tile([P, S], bf16)
        VT_sb = singles.tile([P, S], bf16)
        Vn_sb = singles.tile([P, NKB, P], bf16)
        OT_sb = singles.tile([P, NH, S], bf16)

        # ---------------- phase 1: projections + RoPE + V transpose ---------
        with tc.tile_pool(name="pj", bufs=4, space="PSUM") as pj, \
             tc.tile_pool(name="rope", bufs=2) as rp, \
             tc.tile_pool(name="vtp", bufs=2, space="PSUM") as vtp:

            # PE warmup during the initial DMA window: no-dep matmuls keep the
            # HAM activity monitor busy so real matmuls start at 2.4 GHz.
            # Issued before chunk-gated projection matmuls (strict PE FIFO), so
            # they fill each DMA wait instead of idling into a re-throttle.
            warm = vtp.tile([P, P], f32, tag="warm", bufs=1)

            def warmup(n):
                for _ in range(n):
                    nc.tensor.matmul(warm[:, :], ident[:], ident[:],
                                     start=True, stop=True)

            warmup(36)

            def proj(w_sb, head, tok):
                ps = pj.tile([P, QTS], f32, tag="proj")
                for c in range(NCH):
                    nc.tensor.matmul(
                        ps[:, :], w_sb[:, c, ts(head, P)], xT_sb[:, c, ts(tok, QTS)],
                        start=(c == 0), stop=(c == NCH - 1))
                return ps

            def proj_streamed(w_sb, fill=0):
                # chunk-outer: all 4 token tiles accumulate as xT chunks land,
                # so the PE starts ~1 chunk after the first DMA instead of
                # waiting for the full xT load.
                pss = [pj.tile([P, QTS], f32, tag="proj", name=f"pstr{t}")
                       for t in range(4)]
                for c in range(NCH):
                    if fill and c >= 2:
                        warmup(fill)
                    for t in range(4):
                        nc.tensor.matmul(
                            pss[t][:, :], w_sb[:, c, :], xT_sb[:, c, ts(t, QTS)],
                            start=(c == 0), stop=(c == NCH - 1))
                return pss

            def rope(ps, out_region, tok):
                qf = rp.tile([P, QTS], f32, tag="qf")
                nc.scalar.copy(qf[:, :], ps[:, :])
                sw = rp.tile([P, QTS], f32, tag="swap")
                nc.sync.dma_start(sw[0:64, :], qf[64:128, :])
                nc.sync.dma_start(sw[64:128, :], qf[0:64, :])
                t1 = rp.tile([P, QTS], f32, tag="t1")
                nc.vector.tensor_mul(t1[:, :], ps[:, :], cos_sb[:, ts(tok, QTS)])
                t2 = rp.tile([P, QTS], f32, tag="t2")
                nc.vector.tensor_mul(t2[:, :], sw[:, :], sr_sb[:, ts(tok, QTS)])
                nc.vector.tensor_add(out_region, t1[:, :], t2[:, :])

            # K and V first (chunk-streamed): phase 2 consumes them first.
            for t, ps in enumerate(proj_streamed(wkT_sb, fill=8)):
                rope(ps, KT_sb[:, ts(t, QTS)], t)
            for t, ps in enumerate(proj_streamed(wvT_sb)):
                nc.vector.tensor_copy(VT_sb[:, ts(t, QTS)], ps[:, :])
                for b in range(4 * t, 4 * t + 4):
                    tp = vtp.tile([P, P], bf16, tag="vt")
                    nc.tensor.transpose(tp[:, :], VT_sb[:, ts(b, P)], ident[:])
                    nc.vector.tensor_copy(Vn_sb[:, b, :], tp[:, :])
            # t-outer so attention column t=0 (all heads) unblocks first
            for t in range(S // QTS):
                for h in range(NH):
                    ps = proj(wqT_sb, h, t)
                    rope(ps, QT_sb[:, h, ts(t, QTS)], t)

        # ------- phase 2 + 3: attention with o_proj interleaved per q column -
        # Attention inner loop is software-pipelined by TWO blocks: rs/osum
        # for block j-1 are emitted after scores for block j, so the exp of
        # block j-1 hides under ~639ns of already-ready PE work (the PE queue
        # is strict FIFO; with depth-1 pipelining the PE stalls ~350ns/block
        # waiting on each exp).
        with tc.tile_pool(name="spp", bufs=2, space="PSUM") as spp, \
             tc.tile_pool(name="opp", bufs=2, space="PSUM") as opp, \
             tc.tile_pool(name="rsp", bufs=2, space="PSUM") as rsp, \
             tc.tile_pool(name="ypp", bufs=2, space="PSUM") as ypp, \
             tc.tile_pool(name="ptp", bufs=4) as ptp, \
             tc.tile_pool(name="yop", bufs=4) as yop, \
             tc.tile_pool(name="nrm", bufs=3) as nrm:
            for t in range(S // QTS):
                for h in range(NH):
                    qs = QTS * t
                    nj = 4 * t + 4          # key blocks for this q tile
                    osum = opp.tile([P, QTS], f32, tag="osum")
                    rs = rsp.tile([1, QTS], f32, tag="rs")
                    pts = {}

                    def rs_osum(j):
                        co = max(0, P * j - qs)
                        pt = pts.pop(j)
                        nc.tensor.matmul(rs[0:1, co:QTS], ones[:], pt[:, co:QTS],
                                         start=(j == 0), stop=(j == nj - 1))
                        nc.tensor.matmul(osum[:, co:QTS], Vn_sb[:, j, :],
                                         pt[:, co:QTS],
                                         start=(j == 0), stop=(j == nj - 1))

                    for j in range(nj):
                        co = max(0, P * j - qs)
                        diag = j >= 4 * t
                        sp = spp.tile([P, QTS], f32, tag="sp")
                        nc.tensor.matmul(
                            sp[:, co:QTS], KT_sb[:, ts(j, P)],
                            QT_sb[:, h, qs + co:qs + QTS],
                            start=True, stop=not diag)
                        if diag:
                            nc.tensor.matmul(sp[:, co:co + P], masknegT[:],
                                             ident[:], start=False, stop=True)
                        pt = ptp.tile([P, QTS], bf16, tag="pt")
                        nc.scalar.activation(pt[:, co:QTS], sp[:, co:QTS], Exp,
                                             scale=SCALE)
                        pts[j] = pt
                        if j >= 1:
                            rs_osum(j - 1)
                    rs_osum(nj - 1)
                    # 1/rowsum via 2-op Newton-Raphson approx (~2 ULP) straight
                    # off PSUM, then gpsimd broadcast across partitions.
                    rsc = nrm.tile([1, QTS], f32, tag="rsc")
                    rrow = nrm.tile([1, QTS], f32, tag="rrow")
                    nc.vector.reciprocal_approx_accurate(rrow[:, :], rs[0:1, :],
                                                         rsc[:, :])
                    recipB = nrm.tile([P, QTS], f32, tag="recipB")
                    nc.gpsimd.partition_broadcast(recipB[:, :], rrow[:, :])
                    nc.vector.tensor_mul(OT_sb[:, h, qs:qs + QTS], osum[:, :],
                                         recipB[:, :])
                # o_proj for the token blocks whose attention column is done
                for tb in range(4 * t, 4 * t + 4):
                    for ho in range(H // QTS):
                        yp = ypp.tile([P, QTS], f32, tag="yp")
                        for h in range(NH):
                            nc.tensor.matmul(yp[:, :], OT_sb[:, h, ts(tb, P)],
                                             woT_sb[:, h, ts(ho, QTS)],
                                             start=(h == 0), stop=(h == NH - 1))
                        yo = yop.tile([P, QTS], bf16, tag="yo")
                        nc.vector.tensor_copy(yo[:, :], yp[:, :])
                        nc.sync.dma_start(y[ts(tb, P), ts(ho, QTS)], yo[:, :])

    nc.compile()
    return nc


_NC_CACHE = None


def _get_nc():
    global _NC_CACHE
    if _NC_CACHE is None:
        _NC_CACHE = build_nc()
    return _NC_CACHE


def make_in_maps(hidden_states, position_ids, wq, wk, wv, wo):
    """Host-side sharding: 8 cores = (batch b = core//4) x (GQA group g = core%4)."""
    in_maps = []
    xTs, coss, srs = {}, {}, {}
    for b in range(2):
        xTs[b] = np.ascontiguousarray(hidden_states[b].T).astype(BF16)
        inv = 1.0 / (10000.0 ** (np.arange(0, P, 2, dtype=np.float64) / P))
        invd = np.concatenate([inv, inv]).astype(np.float64)
        fr = invd[:, None] * position_ids[b].astype(np.float64)[None, :]
        coss[b] = np.cos(fr).astype(np.float32)
        sr = np.sin(fr).astype(np.float32)
        sr[:64] *= -1.0
        srs[b] = sr
    shards = {}
    for g in range(4):
        shards[g] = dict(
            wqT=np.ascontiguousarray(wq[DQ * g:DQ * (g + 1)].T).astype(BF16),
            wkT=np.ascontiguousarray(wk[P * g:P * (g + 1)].T).astype(BF16),
            wvT=np.ascontiguousarray(wv[P * g:P * (g + 1)].T).astype(BF16),
            woT=np.ascontiguousarray(wo[:, DQ * g:DQ * (g + 1)].T).astype(BF16),
        )
    for core in range(8):
        b, g = core // 4, core % 4
        in_maps.append(dict(xT=xTs[b], cosT=coss[b], sinrotT=srs[b], **shards[g]))
    return in_maps


def kernel(hidden_states, position_ids, wq, wk, wv, wo, **run_kwargs):
    nc = _get_nc()
    in_maps = make_in_maps(np.asarray(hidden_states), np.asarray(position_ids),
                           np.asarray(wq), np.asarray(wk), np.asarray(wv),
                           np.asarray(wo))
    res = run_bass_kernel_spmd(nc, in_maps, core_ids=list(range(8)), **run_kwargs)
    out = np.zeros((2, S, H), np.float32)
    for core in range(8):
        out[core // 4] += res.results[core]["y"].astype(np.float32)
    if run_kwargs:
        kernel.last_results = res
    return out



# revision 14
# speedup vs baseline: 1.1811x; 1.0484x over previous
"""GQA multi-head attention (B=2, S=2048, H=2048, 16 Q heads / 4 KV heads, RoPE,
causal) on 8 Trainium2 NeuronCores.

Sharding: tensor-parallel over GQA groups (4 groups, each 4 Q heads + 1 KV head)
x data-parallel over batch (2). Core c handles batch b = c // 4, group g = c % 4.
Column-parallel q/k/v projections, row-parallel o_proj; the 4 partial o_proj
outputs per batch are summed on the host.

Per-core kernel (all matmuls bf16 with fp32 PSUM accumulation):
  phase 1: Q^T/K^T/V^T projections from X^T, RoPE on Q/K (half-swap done with a
           PSUM->SBUF DMA partition swap), V transposed to natural layout on PE.
  phase 2: flash-style attention in S^T (keys x queries) layout: S^T = K^T.T Q^T
           per 128-key block, exp on ScalarE (no max subtraction - scores are
           bounded), causal mask via a 128x128 0/1 multiply on the diagonal
           blocks, key-axis row sums via a ones-vector matmul, O^T = V^T P^T
           accumulated in PSUM, normalized by 1/rowsum at the end.
  phase 3: row-parallel o_proj Y = O^T.T @ Wo_g^T accumulated over the 4 local
           heads, DMA'd straight from PSUM to HBM.
"""

import sys

for _p in ("/root/.axon_site", "/root/.axon_site/_ro/trn_rl_repo",
           "/root/.axon_site/_ro/pypackages", "/opt/trn_rl_repo"):
    if _p not in sys.path:
        sys.path.append(_p)

import numpy as np
import ml_dtypes

import concourse.bass as bass
import concourse.tile as tile
import concourse.mybir as mybir
from concourse import bacc
from concourse.bass import ts
from concourse.bass_utils import run_bass_kernel_spmd
from concourse.masks import make_identity, make_upper_triangular
from contextlib import ExitStack

BF16 = ml_dtypes.bfloat16
P = 128
S = 2048
H = 2048
NH = 4          # Q heads per core
DQ = NH * P     # 512
NCH = H // P    # 16 hidden chunks
NKB = S // P    # 16 key blocks
QTS = 512       # query tile (phase 2)
SCALE = 1.0 / float(np.sqrt(128.0))


def build_nc():
    f32 = mybir.dt.float32
    bf16 = mybir.dt.bfloat16
    nc = bacc.Bacc("TRN2", target_bir_lowering=False, debug=False)

    xT = nc.dram_tensor("xT", (H, S), bf16, kind="ExternalInput").ap()
    wqT = nc.dram_tensor("wqT", (H, DQ), bf16, kind="ExternalInput").ap()
    wkT = nc.dram_tensor("wkT", (H, P), bf16, kind="ExternalInput").ap()
    wvT = nc.dram_tensor("wvT", (H, P), bf16, kind="ExternalInput").ap()
    woT = nc.dram_tensor("woT", (DQ, H), bf16, kind="ExternalInput").ap()
    cosT = nc.dram_tensor("cosT", (P, S), f32, kind="ExternalInput").ap()
    srT = nc.dram_tensor("sinrotT", (P, S), f32, kind="ExternalInput").ap()
    y = nc.dram_tensor("y", (S, H), bf16, kind="ExternalOutput").ap()
    rss2 = nc.dram_tensor("rss2", (NH * S // QTS, QTS), f32).ap()  # recip scratch

    Exp = mybir.ActivationFunctionType.Exp

    with ExitStack() as ctx:
        tc = ctx.enter_context(tile.TileContext(nc))
        singles = ctx.enter_context(tc.tile_pool(name="singles", bufs=1))

        # Batched input loads: one strided DMA per group-of-chunks (per-DMA
        # issue on the Sync queue costs ~0.6us; 54 small loads would delay the
        # first matmul by ~30us). K/V weights first so the chunk-streamed K/V
        # projections start immediately.
        xT_sb = singles.tile([P, NCH, S], bf16)
        wqT_sb = singles.tile([P, NCH, DQ], bf16)
        wkT_sb = singles.tile([P, NCH, P], bf16)
        wvT_sb = singles.tile([P, NCH, P], bf16)
        xTr = xT.rearrange("(c p) s -> p c s", p=P)
        wqTr = wqT.rearrange("(c p) m -> p c m", p=P)
        cos_sb = singles.tile([P, S], f32)
        sr_sb = singles.tile([P, S], f32)
        nc.sync.dma_start(wkT_sb[:, :, :], wkT.rearrange("(c p) m -> p c m", p=P))
        for c in range(2):  # first chunks individually: K proj streams earliest
            nc.sync.dma_start(xT_sb[:, c, :], xTr[:, c, :])
        nc.sync.dma_start(cos_sb, cosT)
        nc.sync.dma_start(sr_sb, srT)
        nc.sync.dma_start(xT_sb[:, 2:4, :], xTr[:, 2:4, :])
        nc.sync.dma_start(wvT_sb[:, :, :], wvT.rearrange("(c p) m -> p c m", p=P))
        nc.sync.dma_start(xT_sb[:, 4:8, :], xTr[:, 4:8, :])
        nc.sync.dma_start(wqT_sb[:, 0:8, :], wqTr[:, 0:8, :])
        nc.sync.dma_start(xT_sb[:, 8:12, :], xTr[:, 8:12, :])
        nc.sync.dma_start(wqT_sb[:, 8:16, :], wqTr[:, 8:16, :])
        nc.sync.dma_start(xT_sb[:, 12:16, :], xTr[:, 12:16, :])
        woT_sb = singles.tile([P, NH, H], bf16)
        nc.sync.dma_start(woT_sb[:, :, :], woT.rearrange("(c p) m -> p c m", p=P))

        # Additive causal mask, applied on the PE: a matmul of masknegT.T @ I
        # accumulated into the scores PSUM adds -1e9 where key > query.
        masknegT = singles.tile([P, P], bf16)
        make_upper_triangular(nc, masknegT[:], val=-1e9, diag=False)
        ident = singles.tile([P, P], bf16)
        make_identity(nc, ident[:])
        ones = singles.tile([P, 1], bf16)
        nc.vector.memset(ones[:], 1.0)

        QT_sb = singles.tile([P, NH, S], bf16)
        KT_sb = singles.tile([P, S], bf16)
        VT_sb = singles.tile([P, S], bf16)
        Vn_sb = singles.tile([P, NKB, P], bf16)
        OT_sb = singles.tile([P, NH, S], bf16)

        # ---------------- phase 1: projections + RoPE + V transpose ---------
        with tc.tile_pool(name="pj", bufs=4, space="PSUM") as pj, \
             tc.tile_pool(name="rope", bufs=2) as rp, \
             tc.tile_pool(name="vtp", bufs=2, space="PSUM") as vtp:

            # PE warmup during the initial DMA window: no-dep matmuls keep the
            # HAM activity monitor busy so real matmuls start at 2.4 GHz.
            # Issued before chunk-gated projection matmuls (strict PE FIFO), so
            # they fill each DMA wait instead of idling into a re-throttle.
            warm = vtp.tile([P, P], f32, tag="warm", bufs=1)

            def warmup(n):
                for _ in range(n):
                    nc.tensor.matmul(warm[:, :], ident[:], ident[:],
                                     start=True, stop=True)

            warmup(36)

            def proj(w_sb, head, tok):
                ps = pj.tile([P, QTS], f32, tag="proj")
                for c in range(NCH):
                    nc.tensor.matmul(
                        ps[:, :], w_sb[:, c, ts(head, P)], xT_sb[:, c, ts(tok, QTS)],
                        start=(c == 0), stop=(c == NCH - 1))
                return ps

            def proj_streamed(w_sb, fill=0):
                # chunk-outer: all 4 token tiles accumulate as xT chunks land,
                # so the PE starts ~1 chunk after the first DMA instead of
                # waiting for the full xT load.
                pss = [pj.tile([P, QTS], f32, tag="proj", name=f"pstr{t}")
                       for t in range(4)]
                for c in range(NCH):
                    if fill and c >= 2:
                        warmup(fill)
                    for t in range(4):
                        nc.tensor.matmul(
                            pss[t][:, :], w_sb[:, c, :], xT_sb[:, c, ts(t, QTS)],
                            start=(c == 0), stop=(c == NCH - 1))
                return pss

            def rope(ps, out_region, tok):
                qf = rp.tile([P, QTS], f32, tag="qf")
                nc.scalar.copy(qf[:, :], ps[:, :])
                sw = rp.tile([P, QTS], f32, tag="swap")
                nc.sync.dma_start(sw[0:64, :], qf[64:128, :])
                nc.sync.dma_start(sw[64:128, :], qf[0:64, :])
                t1 = rp.tile([P, QTS], f32, tag="t1")
                nc.vector.tensor_mul(t1[:, :], ps[:, :], cos_sb[:, ts(tok, QTS)])
                t2 = rp.tile([P, QTS], f32, tag="t2")
                nc.vector.tensor_mul(t2[:, :], sw[:, :], sr_sb[:, ts(tok, QTS)])
                nc.vector.tensor_add(out_region, t1[:, :], t2[:, :])

            # K and V first (chunk-streamed): phase 2 consumes them first.
            for t, ps in enumerate(proj_streamed(wkT_sb, fill=8)):
                rope(ps, KT_sb[:, ts(t, QTS)], t)
            for t, ps in enumerate(proj_streamed(wvT_sb)):
                nc.vector.tensor_copy(VT_sb[:, ts(t, QTS)], ps[:, :])
                for b in range(4 * t, 4 * t + 4):
                    tp = vtp.tile([P, P], bf16, tag="vt")
                    nc.tensor.transpose(tp[:, :], VT_sb[:, ts(b, P)], ident[:])
                    nc.vector.tensor_copy(Vn_sb[:, b, :], tp[:, :])
            # t-outer so attention column t=0 (all heads) unblocks first
            for t in range(S // QTS):
                for h in range(NH):
                    ps = proj(wqT_sb, h, t)
                    rope(ps, QT_sb[:, h, ts(t, QTS)], t)

        # ------- phase 2 + 3: attention with o_proj interleaved per q column -
        # Key blocks are processed in PAIRS sharing one [128, 2, 512] PSUM
        # scores tile so each exp ACTIVATE covers ~1024 elems/partition: the
        # 352-cycle fixed overhead per ACTIVATE made per-block exp (687ns)
        # slower than the 642ns of PE work per block - attention was
        # scalar-bound.  Paired, exp is ~1147ns vs ~1284ns PE work per pair.
        # rs/osum for pair p-1 are emitted after scores+exp of pair p, so each
        # exp hides under already-ready PE work (PE queue is strict FIFO).
        # osum / rowsum / o_proj accumulators share one 4-buffer PSUM ring
        # ("acc") to fit: 2*2 (sp) + 4 (acc) = 8 banks.
        with tc.tile_pool(name="sp2", bufs=2, space="PSUM") as sp2, \
             tc.tile_pool(name="accp", bufs=4, space="PSUM") as accp, \
             tc.tile_pool(name="ptp", bufs=3) as ptp, \
             tc.tile_pool(name="yop", bufs=4) as yop, \
             tc.tile_pool(name="nrm", bufs=3) as nrm:
            for t in range(S // QTS):
                for h in range(NH):
                    qs = QTS * t
                    nj = 4 * t + 4          # key blocks for this q tile
                    osum = accp.tile([P, QTS], f32, tag="acc", name="osum")
                    rs = accp.tile([P, QTS], f32, tag="acc", name="rs")
                    npair = nj // 2

                    def rs_osum(p, pt):
                        for jj in range(2):
                            j = 2 * p + jj
                            co = max(0, P * j - qs)
                            nc.tensor.matmul(rs[0:1, co:QTS], ones[:],
                                             pt[:, jj, co:QTS],
                                             start=(j == 0), stop=(j == nj - 1))
                            nc.tensor.matmul(osum[:, co:QTS], Vn_sb[:, j, :],
                                             pt[:, jj, co:QTS],
                                             start=(j == 0), stop=(j == nj - 1))

                    pend = None
                    for pr in range(npair):
                        sp = sp2.tile([P, 2, QTS], f32, tag="sp")
                        co0 = max(0, P * 2 * pr - qs)
                        for jj in range(2):
                            j = 2 * pr + jj
                            co = max(0, P * j - qs)
                            diag = j >= 4 * t
                            nc.tensor.matmul(
                                sp[:, jj, co:QTS], KT_sb[:, ts(j, P)],
                                QT_sb[:, h, qs + co:qs + QTS],
                                start=True, stop=not diag)
                            if diag:
                                nc.tensor.matmul(sp[:, jj, co:co + P],
                                                 masknegT[:], ident[:],
                                                 start=False, stop=True)
                        pt = ptp.tile([P, 2, QTS], bf16, tag="pt")
                        nc.scalar.activation(pt[:, :, co0:QTS],
                                             sp[:, :, co0:QTS], Exp,
                                             scale=SCALE)
                        if pend is not None:
                            rs_osum(*pend)
                        pend = (pr, pt)
                    rs_osum(*pend)
                    # 1/rowsum via 2-op Newton-Raphson approx (~2 ULP) straight
                    # off PSUM, then gpsimd broadcast across partitions.
                    rsc = nrm.tile([1, QTS], f32, tag="rsc")
                    rrow = nrm.tile([1, QTS], f32, tag="rrow")
                    nc.vector.reciprocal_approx_accurate(rrow[:, :], rs[0:1, :],
                                                         rsc[:, :])
                    recipB = nrm.tile([P, QTS], f32, tag="recipB")
                    nc.gpsimd.partition_broadcast(recipB[:, :], rrow[:, :])
                    nc.vector.tensor_mul(OT_sb[:, h, qs:qs + QTS], osum[:, :],
                                         recipB[:, :])
                # o_proj for the token blocks whose attention column is done
                for tb in range(4 * t, 4 * t + 4):
                    for ho in range(H // QTS):
                        yp = accp.tile([P, QTS], f32, tag="acc", name="yp")
                        for h in range(NH):
                            nc.tensor.matmul(yp[:, :], OT_sb[:, h, ts(tb, P)],
                                             woT_sb[:, h, ts(ho, QTS)],
                                             start=(h == 0), stop=(h == NH - 1))
                        yo = yop.tile([P, QTS], bf16, tag="yo")
                        nc.vector.tensor_copy(yo[:, :], yp[:, :])
                        nc.sync.dma_start(y[ts(tb, P), ts(ho, QTS)], yo[:, :])

    nc.compile()
    return nc


_NC_CACHE = None


def _get_nc():
    global _NC_CACHE
    if _NC_CACHE is None:
        _NC_CACHE = build_nc()
    return _NC_CACHE


def make_in_maps(hidden_states, position_ids, wq, wk, wv, wo):
    """Host-side sharding: 8 cores = (batch b = core//4) x (GQA group g = core%4)."""
    in_maps = []
    xTs, coss, srs = {}, {}, {}
    for b in range(2):
        xTs[b] = np.ascontiguousarray(hidden_states[b].T).astype(BF16)
        inv = 1.0 / (10000.0 ** (np.arange(0, P, 2, dtype=np.float64) / P))
        invd = np.concatenate([inv, inv]).astype(np.float64)
        fr = invd[:, None] * position_ids[b].astype(np.float64)[None, :]
        coss[b] = np.cos(fr).astype(np.float32)
        sr = np.sin(fr).astype(np.float32)
        sr[:64] *= -1.0
        srs[b] = sr
    shards = {}
    for g in range(4):
        shards[g] = dict(
            wqT=np.ascontiguousarray(wq[DQ * g:DQ * (g + 1)].T).astype(BF16),
            wkT=np.ascontiguousarray(wk[P * g:P * (g + 1)].T).astype(BF16),
            wvT=np.ascontiguousarray(wv[P * g:P * (g + 1)].T).astype(BF16),
            woT=np.ascontiguousarray(wo[:, DQ * g:DQ * (g + 1)].T).astype(BF16),
        )
    for core in range(8):
        b, g = core // 4, core % 4
        in_maps.append(dict(xT=xTs[b], cosT=coss[b], sinrotT=srs[b], **shards[g]))
    return in_maps


def kernel(hidden_states, position_ids, wq, wk, wv, wo, **run_kwargs):
    nc = _get_nc()
    in_maps = make_in_maps(np.asarray(hidden_states), np.asarray(position_ids),
                           np.asarray(wq), np.asarray(wk), np.asarray(wv),
                           np.asarray(wo))
    res = run_bass_kernel_spmd(nc, in_maps, core_ids=list(range(8)), **run_kwargs)
    out = np.zeros((2, S, H), np.float32)
    for core in range(8):
        out[core // 4] += res.results[core]["y"].astype(np.float32)
    if run_kwargs:
        kernel.last_results = res
    return out



# revision 17
# speedup vs baseline: 1.3151x; 1.1134x over previous
"""GQA multi-head attention (B=2, S=2048, H=2048, 16 Q heads / 4 KV heads, RoPE,
causal) on 8 Trainium2 NeuronCores.

Sharding: tensor-parallel over GQA groups (4 groups, each 4 Q heads + 1 KV head)
x data-parallel over batch (2). Core c handles batch b = c // 4, group g = c % 4.
Column-parallel q/k/v projections, row-parallel o_proj; the 4 partial o_proj
outputs per batch are summed on the host.

Per-core kernel (all matmuls bf16 with fp32 PSUM accumulation):
  phase 1: Q^T/K^T/V^T projections from X^T, RoPE on Q/K (half-swap done with a
           PSUM->SBUF DMA partition swap), V transposed to natural layout on PE.
  phase 2: flash-style attention in S^T (keys x queries) layout: S^T = K^T.T Q^T
           per 128-key block, exp on ScalarE (no max subtraction - scores are
           bounded), causal mask via a 128x128 0/1 multiply on the diagonal
           blocks, key-axis row sums via a ones-vector matmul, O^T = V^T P^T
           accumulated in PSUM, normalized by 1/rowsum at the end.
  phase 3: row-parallel o_proj Y = O^T.T @ Wo_g^T accumulated over the 4 local
           heads, DMA'd straight from PSUM to HBM.
"""

import sys

for _p in ("/root/.axon_site", "/root/.axon_site/_ro/trn_rl_repo",
           "/root/.axon_site/_ro/pypackages", "/opt/trn_rl_repo"):
    if _p not in sys.path:
        sys.path.append(_p)

import numpy as np
import ml_dtypes

import concourse.bass as bass
import concourse.tile as tile
import concourse.mybir as mybir
from concourse import bacc
from concourse.bass import ts
from concourse.bass_utils import run_bass_kernel_spmd
from concourse.masks import make_identity, make_upper_triangular
from contextlib import ExitStack

BF16 = ml_dtypes.bfloat16
P = 128
S = 2048
H = 2048
NH = 4          # Q heads per core
DQ = NH * P     # 512
NCH = H // P    # 16 hidden chunks
NKB = S // P    # 16 key blocks
QTS = 512       # query tile (phase 2)
SCALE = 1.0 / float(np.sqrt(128.0))


def build_nc():
    f32 = mybir.dt.float32
    bf16 = mybir.dt.bfloat16
    nc = bacc.Bacc("TRN2", target_bir_lowering=False, debug=False)

    xT = nc.dram_tensor("xT", (H, S), bf16, kind="ExternalInput").ap()
    wqT = nc.dram_tensor("wqT", (H, DQ), bf16, kind="ExternalInput").ap()
    wkT = nc.dram_tensor("wkT", (H, P), bf16, kind="ExternalInput").ap()
    wvT = nc.dram_tensor("wvT", (H, P), bf16, kind="ExternalInput").ap()
    woT = nc.dram_tensor("woT", (DQ, H), bf16, kind="ExternalInput").ap()
    cosT = nc.dram_tensor("cosT", (P, S), f32, kind="ExternalInput").ap()
    srT = nc.dram_tensor("sinrotT", (P, S), f32, kind="ExternalInput").ap()
    y = nc.dram_tensor("y", (S, H), bf16, kind="ExternalOutput").ap()
    rss2 = nc.dram_tensor("rss2", (NH * S // QTS, QTS), f32).ap()  # recip scratch

    Exp = mybir.ActivationFunctionType.Exp

    with ExitStack() as ctx:
        tc = ctx.enter_context(tile.TileContext(nc))
        singles = ctx.enter_context(tc.tile_pool(name="singles", bufs=1))

        # Batched input loads: one strided DMA per group-of-chunks (per-DMA
        # issue on the Sync queue costs ~0.6us; 54 small loads would delay the
        # first matmul by ~30us). K/V weights first so the chunk-streamed K/V
        # projections start immediately.
        xT_sb = singles.tile([P, NCH, S], bf16)
        wqT_sb = singles.tile([P, NCH, DQ], bf16)
        wkT_sb = singles.tile([P, NCH, P], bf16)
        wvT_sb = singles.tile([P, NCH, P], bf16)
        xTr = xT.rearrange("(c p) s -> p c s", p=P)
        wqTr = wqT.rearrange("(c p) m -> p c m", p=P)
        cos_sb = singles.tile([P, S], f32)
        sr_sb = singles.tile([P, S], f32)
        nc.sync.dma_start(wkT_sb[:, :, :], wkT.rearrange("(c p) m -> p c m", p=P))
        for c in range(2):  # first chunks individually: K proj streams earliest
            nc.sync.dma_start(xT_sb[:, c, :], xTr[:, c, :])
        nc.sync.dma_start(cos_sb, cosT)
        nc.sync.dma_start(sr_sb, srT)
        nc.sync.dma_start(xT_sb[:, 2:4, :], xTr[:, 2:4, :])
        nc.sync.dma_start(wvT_sb[:, :, :], wvT.rearrange("(c p) m -> p c m", p=P))
        nc.sync.dma_start(xT_sb[:, 4:8, :], xTr[:, 4:8, :])
        nc.sync.dma_start(wqT_sb[:, 0:8, :], wqTr[:, 0:8, :])
        nc.sync.dma_start(xT_sb[:, 8:12, :], xTr[:, 8:12, :])
        nc.sync.dma_start(wqT_sb[:, 8:16, :], wqTr[:, 8:16, :])
        nc.sync.dma_start(xT_sb[:, 12:16, :], xTr[:, 12:16, :])
        woT_sb = singles.tile([P, NH, H], bf16)
        nc.sync.dma_start(woT_sb[:, :, :], woT.rearrange("(c p) m -> p c m", p=P))

        # Additive causal mask, applied on the PE: a matmul of masknegT.T @ I
        # accumulated into the scores PSUM adds -1e9 where key > query.
        masknegT = singles.tile([P, P], bf16)
        make_upper_triangular(nc, masknegT[:], val=-1e9, diag=False)
        ident = singles.tile([P, P], bf16)
        make_identity(nc, ident[:])
        # Full [128,128] ones: the rowsum matmul then uses all PE column
        # groups (an M=1 matmul runs col_grp-restricted, and switching
        # col_grp between matmuls breaks back-to-back pipelining: 309ns
        # vs 216ns spacing), and every PSUM partition gets the rowsum,
        # which doubles as the partition-broadcast for the normalizer.
        ones = singles.tile([P, P], bf16)
        nc.vector.memset(ones[:], 1.0)

        QT_sb = singles.tile([P, NH, S], bf16)
        KT_sb = singles.tile([P, S], bf16)
        VT_sb = singles.tile([P, S], bf16)
        Vn_sb = singles.tile([P, NKB, P], bf16)
        OT_sb = singles.tile([P, NH, S], bf16)

        # ---------------- phase 1: projections + RoPE + V transpose ---------
        with tc.tile_pool(name="pj", bufs=4, space="PSUM") as pj, \
             tc.tile_pool(name="rope", bufs=2) as rp, \
             tc.tile_pool(name="vtp", bufs=2, space="PSUM") as vtp:

            # PE warmup during the initial DMA window: no-dep matmuls keep the
            # HAM activity monitor busy so real matmuls start at 2.4 GHz.
            # Issued before chunk-gated projection matmuls (strict PE FIFO), so
            # they fill each DMA wait instead of idling into a re-throttle.
            warm = vtp.tile([P, P], f32, tag="warm", bufs=1)

            def warmup(n):
                for _ in range(n):
                    nc.tensor.matmul(warm[:, :], ident[:], ident[:],
                                     start=True, stop=True)

            warmup(36)

            def proj(w_sb, head, tok):
                ps = pj.tile([P, QTS], f32, tag="proj")
                for c in range(NCH):
                    nc.tensor.matmul(
                        ps[:, :], w_sb[:, c, ts(head, P)], xT_sb[:, c, ts(tok, QTS)],
                        start=(c == 0), stop=(c == NCH - 1))
                return ps

            def proj_streamed(w_sb, fill=0):
                # chunk-outer: all 4 token tiles accumulate as xT chunks land,
                # so the PE starts ~1 chunk after the first DMA instead of
                # waiting for the full xT load.
                pss = [pj.tile([P, QTS], f32, tag="proj", name=f"pstr{t}")
                       for t in range(4)]
                for c in range(NCH):
                    if fill and c >= 2:
                        warmup(fill)
                    for t in range(4):
                        nc.tensor.matmul(
                            pss[t][:, :], w_sb[:, c, :], xT_sb[:, c, ts(t, QTS)],
                            start=(c == 0), stop=(c == NCH - 1))
                return pss

            def rope(ps, out_region, tok):
                qf = rp.tile([P, QTS], f32, tag="qf")
                nc.scalar.copy(qf[:, :], ps[:, :])
                sw = rp.tile([P, QTS], f32, tag="swap")
                nc.sync.dma_start(sw[0:64, :], qf[64:128, :])
                nc.sync.dma_start(sw[64:128, :], qf[0:64, :])
                t1 = rp.tile([P, QTS], f32, tag="t1")
                nc.vector.tensor_mul(t1[:, :], ps[:, :], cos_sb[:, ts(tok, QTS)])
                t2 = rp.tile([P, QTS], f32, tag="t2")
                nc.vector.tensor_mul(t2[:, :], sw[:, :], sr_sb[:, ts(tok, QTS)])
                nc.vector.tensor_add(out_region, t1[:, :], t2[:, :])

            # K and V first (chunk-streamed): phase 2 consumes them first.
            for t, ps in enumerate(proj_streamed(wkT_sb, fill=8)):
                rope(ps, KT_sb[:, ts(t, QTS)], t)
            for t, ps in enumerate(proj_streamed(wvT_sb)):
                nc.vector.tensor_copy(VT_sb[:, ts(t, QTS)], ps[:, :])
                for b in range(4 * t, 4 * t + 4):
                    tp = vtp.tile([P, P], bf16, tag="vt")
                    nc.tensor.transpose(tp[:, :], VT_sb[:, ts(b, P)], ident[:])
                    nc.vector.tensor_copy(Vn_sb[:, b, :], tp[:, :])
            # t-outer so attention column t=0 (all heads) unblocks first
            for t in range(S // QTS):
                for h in range(NH):
                    ps = proj(wqT_sb, h, t)
                    rope(ps, QT_sb[:, h, ts(t, QTS)], t)

        # ------- phase 2 + 3: attention with o_proj interleaved per q column -
        # Key blocks are processed in PAIRS sharing one [128, 2, 512] PSUM
        # scores tile so each exp ACTIVATE covers ~1024 elems/partition: the
        # 352-cycle fixed overhead per ACTIVATE made per-block exp (687ns)
        # slower than the 642ns of PE work per block - attention was
        # scalar-bound.  Paired, exp is ~1147ns vs ~1284ns PE work per pair.
        # rs/osum for pair p-1 are emitted after scores+exp of pair p, so each
        # exp hides under already-ready PE work (PE queue is strict FIFO).
        # osum / rowsum / o_proj accumulators share one 4-buffer PSUM ring
        # ("acc") to fit: 2*2 (sp) + 4 (acc) = 8 banks.
        with tc.tile_pool(name="sp2", bufs=2, space="PSUM") as sp2, \
             tc.tile_pool(name="accp", bufs=4, space="PSUM") as accp, \
             tc.tile_pool(name="ptp", bufs=3) as ptp, \
             tc.tile_pool(name="yop", bufs=4) as yop, \
             tc.tile_pool(name="nrm", bufs=3) as nrm:
            for t in range(S // QTS):
                for h in range(NH):
                    qs = QTS * t
                    nj = 4 * t + 4          # key blocks for this q tile
                    osum = accp.tile([P, QTS], f32, tag="acc", name="osum")
                    rs = accp.tile([P, QTS], f32, tag="acc", name="rs")
                    npair = nj // 2

                    def rs_osum(p, pt):
                        for jj in range(2):
                            j = 2 * p + jj
                            co = max(0, P * j - qs)
                            nc.tensor.matmul(rs[:, co:QTS], ones[:],
                                             pt[:, jj, co:QTS],
                                             start=(j == 0), stop=(j == nj - 1))
                            nc.tensor.matmul(osum[:, co:QTS], Vn_sb[:, j, :],
                                             pt[:, jj, co:QTS],
                                             start=(j == 0), stop=(j == nj - 1))

                    pend = None
                    for pr in range(npair):
                        sp = sp2.tile([P, 2, QTS], f32, tag="sp")
                        co0 = max(0, P * 2 * pr - qs)
                        for jj in range(2):
                            j = 2 * pr + jj
                            co = max(0, P * j - qs)
                            diag = j >= 4 * t
                            nc.tensor.matmul(
                                sp[:, jj, co:QTS], KT_sb[:, ts(j, P)],
                                QT_sb[:, h, qs + co:qs + QTS],
                                start=True, stop=not diag)
                            if diag:
                                nc.tensor.matmul(sp[:, jj, co:co + P],
                                                 masknegT[:], ident[:],
                                                 start=False, stop=True)
                        pt = ptp.tile([P, 2, QTS], bf16, tag="pt")
                        nc.scalar.activation(pt[:, :, co0:QTS],
                                             sp[:, :, co0:QTS], Exp,
                                             scale=SCALE)
                        if pend is not None:
                            rs_osum(*pend)
                        pend = (pr, pt)
                    rs_osum(*pend)
                    # 1/rowsum via 2-op Newton-Raphson approx (~2 ULP) straight
                    # off PSUM (already partition-broadcast by the ones matmul).
                    rsc = nrm.tile([P, QTS], f32, tag="rsc")
                    recipB = nrm.tile([P, QTS], f32, tag="recipB")
                    nc.vector.reciprocal_approx_accurate(recipB[:, :], rs[:, :],
                                                         rsc[:, :])
                    nc.vector.tensor_mul(OT_sb[:, h, qs:qs + QTS], osum[:, :],
                                         recipB[:, :])
                # o_proj for the token blocks whose attention column is done
                for tb in range(4 * t, 4 * t + 4):
                    for ho in range(H // QTS):
                        yp = accp.tile([P, QTS], f32, tag="acc", name="yp")
                        for h in range(NH):
                            nc.tensor.matmul(yp[:, :], OT_sb[:, h, ts(tb, P)],
                                             woT_sb[:, h, ts(ho, QTS)],
                                             start=(h == 0), stop=(h == NH - 1))
                        yo = yop.tile([P, QTS], bf16, tag="yo")
                        nc.vector.tensor_copy(yo[:, :], yp[:, :])
                        nc.sync.dma_start(y[ts(tb, P), ts(ho, QTS)], yo[:, :])

    nc.compile()
    return nc


_NC_CACHE = None


def _get_nc():
    global _NC_CACHE
    if _NC_CACHE is None:
        _NC_CACHE = build_nc()
    return _NC_CACHE


def make_in_maps(hidden_states, position_ids, wq, wk, wv, wo):
    """Host-side sharding: 8 cores = (batch b = core//4) x (GQA group g = core%4)."""
    in_maps = []
    xTs, coss, srs = {}, {}, {}
    for b in range(2):
        xTs[b] = np.ascontiguousarray(hidden_states[b].T).astype(BF16)
        inv = 1.0 / (10000.0 ** (np.arange(0, P, 2, dtype=np.float64) / P))
        invd = np.concatenate([inv, inv]).astype(np.float64)
        fr = invd[:, None] * position_ids[b].astype(np.float64)[None, :]
        coss[b] = np.cos(fr).astype(np.float32)
        sr = np.sin(fr).astype(np.float32)
        sr[:64] *= -1.0
        srs[b] = sr
    shards = {}
    for g in range(4):
        shards[g] = dict(
            wqT=np.ascontiguousarray(wq[DQ * g:DQ * (g + 1)].T).astype(BF16),
            wkT=np.ascontiguousarray(wk[P * g:P * (g + 1)].T).astype(BF16),
            wvT=np.ascontiguousarray(wv[P * g:P * (g + 1)].T).astype(BF16),
            woT=np.ascontiguousarray(wo[:, DQ * g:DQ * (g + 1)].T).astype(BF16),
        )
    for core in range(8):
        b, g = core // 4, core % 4
        in_maps.append(dict(xT=xTs[b], cosT=coss[b], sinrotT=srs[b], **shards[g]))
    return in_maps


def kernel(hidden_states, position_ids, wq, wk, wv, wo, **run_kwargs):
    nc = _get_nc()
    in_maps = make_in_maps(np.asarray(hidden_states), np.asarray(position_ids),
                           np.asarray(wq), np.asarray(wk), np.asarray(wv),
                           np.asarray(wo))
    res = run_bass_kernel_spmd(nc, in_maps, core_ids=list(range(8)), **run_kwargs)
    out = np.zeros((2, S, H), np.float32)
    for core in range(8):
        out[core // 4] += res.results[core]["y"].astype(np.float32)
    if run_kwargs:
        kernel.last_results = res
    return out



# revision 20
# speedup vs baseline: 1.3246x; 1.0072x over previous
"""GQA multi-head attention (B=2, S=2048, H=2048, 16 Q heads / 4 KV heads, RoPE,
causal) on 8 Trainium2 NeuronCores.

Sharding: tensor-parallel over GQA groups (4 groups, each 4 Q heads + 1 KV head)
x data-parallel over batch (2). Core c handles batch b = c // 4, group g = c % 4.
Column-parallel q/k/v projections, row-parallel o_proj; the 4 partial o_proj
outputs per batch are summed on the host.

Per-core kernel (all matmuls bf16 with fp32 PSUM accumulation):
  phase 1: Q^T/K^T/V^T projections from X^T, RoPE on Q/K (half-swap done with a
           PSUM->SBUF DMA partition swap), V transposed to natural layout on PE.
  phase 2: flash-style attention in S^T (keys x queries) layout: S^T = K^T.T Q^T
           per 128-key block, exp on ScalarE (no max subtraction - scores are
           bounded), causal mask via a 128x128 0/1 multiply on the diagonal
           blocks, key-axis row sums via a ones-vector matmul, O^T = V^T P^T
           accumulated in PSUM, normalized by 1/rowsum at the end.
  phase 3: row-parallel o_proj Y = O^T.T @ Wo_g^T accumulated over the 4 local
           heads, DMA'd straight from PSUM to HBM.
"""

import sys

for _p in ("/root/.axon_site", "/root/.axon_site/_ro/trn_rl_repo",
           "/root/.axon_site/_ro/pypackages", "/opt/trn_rl_repo"):
    if _p not in sys.path:
        sys.path.append(_p)

import numpy as np
import ml_dtypes

import concourse.bass as bass
import concourse.tile as tile
import concourse.mybir as mybir
from concourse import bacc
from concourse.bass import ts
from concourse.bass_utils import run_bass_kernel_spmd
from concourse.masks import make_identity, make_upper_triangular
from contextlib import ExitStack

BF16 = ml_dtypes.bfloat16
P = 128
S = 2048
H = 2048
NH = 4          # Q heads per core
DQ = NH * P     # 512
NCH = H // P    # 16 hidden chunks
NKB = S // P    # 16 key blocks
QTS = 512       # query tile (phase 2)
SCALE = 1.0 / float(np.sqrt(128.0))


def build_nc():
    f32 = mybir.dt.float32
    bf16 = mybir.dt.bfloat16
    nc = bacc.Bacc("TRN2", target_bir_lowering=False, debug=False)

    xT = nc.dram_tensor("xT", (H, S), bf16, kind="ExternalInput").ap()
    wqT = nc.dram_tensor("wqT", (H, DQ), bf16, kind="ExternalInput").ap()
    wkT = nc.dram_tensor("wkT", (H, P), bf16, kind="ExternalInput").ap()
    wvT = nc.dram_tensor("wvT", (H, P), bf16, kind="ExternalInput").ap()
    woT = nc.dram_tensor("woT", (DQ, H), bf16, kind="ExternalInput").ap()
    cosT = nc.dram_tensor("cosT", (P, S), bf16, kind="ExternalInput").ap()
    srT = nc.dram_tensor("sinrotT", (P, S), bf16, kind="ExternalInput").ap()
    y = nc.dram_tensor("y", (S, H), bf16, kind="ExternalOutput").ap()
    rss2 = nc.dram_tensor("rss2", (NH * S // QTS, QTS), f32).ap()  # recip scratch

    Exp = mybir.ActivationFunctionType.Exp

    with ExitStack() as ctx:
        tc = ctx.enter_context(tile.TileContext(nc))
        singles = ctx.enter_context(tc.tile_pool(name="singles", bufs=1))

        # Batched input loads: one strided DMA per group-of-chunks (per-DMA
        # issue on the Sync queue costs ~0.6us; 54 small loads would delay the
        # first matmul by ~30us). K/V weights first so the chunk-streamed K/V
        # projections start immediately.
        xT_sb = singles.tile([P, NCH, S], bf16)
        wqT_sb = singles.tile([P, NCH, DQ], bf16)
        wkT_sb = singles.tile([P, NCH, P], bf16)
        wvT_sb = singles.tile([P, NCH, P], bf16)
        xTr = xT.rearrange("(c p) s -> p c s", p=P)
        wqTr = wqT.rearrange("(c p) m -> p c m", p=P)
        cos_sb = singles.tile([P, S], bf16)
        sr_sb = singles.tile([P, S], bf16)
        nc.sync.dma_start(wkT_sb[:, :, :], wkT.rearrange("(c p) m -> p c m", p=P))
        for c in range(2):  # first chunks individually: K proj streams earliest
            nc.sync.dma_start(xT_sb[:, c, :], xTr[:, c, :])
        nc.sync.dma_start(cos_sb, cosT)
        nc.sync.dma_start(sr_sb, srT)
        nc.sync.dma_start(wvT_sb[:, :, :], wvT.rearrange("(c p) m -> p c m", p=P))
        for c in range(2, 8):
            nc.sync.dma_start(xT_sb[:, c, :], xTr[:, c, :])
        nc.sync.dma_start(wqT_sb[:, 0:8, :], wqTr[:, 0:8, :])
        for c in range(8, 12):
            nc.sync.dma_start(xT_sb[:, c, :], xTr[:, c, :])
        nc.sync.dma_start(wqT_sb[:, 8:16, :], wqTr[:, 8:16, :])
        for c in range(12, 16):
            nc.sync.dma_start(xT_sb[:, c, :], xTr[:, c, :])
        woT_sb = singles.tile([P, NH, H], bf16)
        nc.sync.dma_start(woT_sb[:, :, :], woT.rearrange("(c p) m -> p c m", p=P))

        # Additive causal mask, applied on the PE: a matmul of masknegT.T @ I
        # accumulated into the scores PSUM adds -1e9 where key > query.
        masknegT = singles.tile([P, P], bf16)
        make_upper_triangular(nc, masknegT[:], val=-1e9, diag=False)
        ident = singles.tile([P, P], bf16)
        make_identity(nc, ident[:])
        # Full [128,128] ones: the rowsum matmul then uses all PE column
        # groups (an M=1 matmul runs col_grp-restricted, and switching
        # col_grp between matmuls breaks back-to-back pipelining: 309ns
        # vs 216ns spacing), and every PSUM partition gets the rowsum,
        # which doubles as the partition-broadcast for the normalizer.
        ones = singles.tile([P, P], bf16)
        nc.vector.memset(ones[:], 1.0)

        QT_sb = singles.tile([P, NH, S], bf16)
        KT_sb = singles.tile([P, S], bf16)
        VT_sb = singles.tile([P, S], bf16)
        Vn_sb = singles.tile([P, NKB, P], bf16)
        OT_sb = singles.tile([P, NH, S], bf16)

        # ---------------- phase 1: projections + RoPE + V transpose ---------
        with tc.tile_pool(name="pj", bufs=4, space="PSUM") as pj, \
             tc.tile_pool(name="rope", bufs=2) as rp, \
             tc.tile_pool(name="vtp", bufs=2, space="PSUM") as vtp:

            # PE warmup during the initial DMA window: no-dep matmuls keep the
            # HAM activity monitor busy so real matmuls start at 2.4 GHz.
            # Issued before chunk-gated projection matmuls (strict PE FIFO), so
            # they fill each DMA wait instead of idling into a re-throttle.
            warm = vtp.tile([P, P], f32, tag="warm", bufs=1)

            def warmup(n):
                for _ in range(n):
                    nc.tensor.matmul(warm[:, :], ident[:], ident[:],
                                     start=True, stop=True)

            warmup(36)

            def proj(w_sb, head, tok):
                ps = pj.tile([P, QTS], f32, tag="proj")
                for c in range(NCH):
                    nc.tensor.matmul(
                        ps[:, :], w_sb[:, c, ts(head, P)], xT_sb[:, c, ts(tok, QTS)],
                        start=(c == 0), stop=(c == NCH - 1))
                return ps

            def proj_streamed(w_sb, fill=0):
                # chunk-outer: all 4 token tiles accumulate as xT chunks land,
                # so the PE starts ~1 chunk after the first DMA instead of
                # waiting for the full xT load.
                pss = [pj.tile([P, QTS], f32, tag="proj", name=f"pstr{t}")
                       for t in range(4)]
                for c in range(NCH):
                    if fill and c >= 2:
                        warmup(fill)
                    for t in range(4):
                        nc.tensor.matmul(
                            pss[t][:, :], w_sb[:, c, :], xT_sb[:, c, ts(t, QTS)],
                            start=(c == 0), stop=(c == NCH - 1))
                return pss

            def rope(ps, out_region, tok):
                qf = rp.tile([P, QTS], f32, tag="qf")
                nc.scalar.copy(qf[:, :], ps[:, :])
                sw = rp.tile([P, QTS], f32, tag="swap")
                nc.sync.dma_start(sw[0:64, :], qf[64:128, :])
                nc.sync.dma_start(sw[64:128, :], qf[0:64, :])
                t1 = rp.tile([P, QTS], f32, tag="t1")
                nc.vector.tensor_mul(t1[:, :], ps[:, :], cos_sb[:, ts(tok, QTS)])
                t2 = rp.tile([P, QTS], f32, tag="t2")
                nc.vector.tensor_mul(t2[:, :], sw[:, :], sr_sb[:, ts(tok, QTS)])
                nc.vector.tensor_add(out_region, t1[:, :], t2[:, :])

            # K and V first (chunk-streamed): phase 2 consumes them first.
            for t, ps in enumerate(proj_streamed(wkT_sb, fill=8)):
                rope(ps, KT_sb[:, ts(t, QTS)], t)
            for t, ps in enumerate(proj_streamed(wvT_sb)):
                nc.vector.tensor_copy(VT_sb[:, ts(t, QTS)], ps[:, :])
                for b in range(4 * t, 4 * t + 4):
                    tp = vtp.tile([P, P], bf16, tag="vt")
                    nc.tensor.transpose(tp[:, :], VT_sb[:, ts(b, P)], ident[:])
                    nc.vector.tensor_copy(Vn_sb[:, b, :], tp[:, :])
            # t-outer so attention column t=0 (all heads) unblocks first
            for t in range(S // QTS):
                for h in range(NH):
                    ps = proj(wqT_sb, h, t)
                    rope(ps, QT_sb[:, h, ts(t, QTS)], t)

        # ------- phase 2 + 3: attention with o_proj interleaved per q column -
        # Key blocks are processed in PAIRS sharing one [128, 2, 512] PSUM
        # scores tile so each exp ACTIVATE covers ~1024 elems/partition: the
        # 352-cycle fixed overhead per ACTIVATE made per-block exp (687ns)
        # slower than the 642ns of PE work per block - attention was
        # scalar-bound.  Paired, exp is ~1147ns vs ~1284ns PE work per pair.
        # rs/osum for pair p-1 are emitted after scores+exp of pair p, so each
        # exp hides under already-ready PE work (PE queue is strict FIFO).
        # osum / rowsum / o_proj accumulators share one 4-buffer PSUM ring
        # ("acc") to fit: 2*2 (sp) + 4 (acc) = 8 banks.
        with tc.tile_pool(name="sp2", bufs=2, space="PSUM") as sp2, \
             tc.tile_pool(name="accp", bufs=4, space="PSUM") as accp, \
             tc.tile_pool(name="ptp", bufs=3) as ptp, \
             tc.tile_pool(name="yop", bufs=4) as yop, \
             tc.tile_pool(name="nrm", bufs=3) as nrm:
            for t in range(S // QTS):
                for h in range(NH):
                    qs = QTS * t
                    nj = 4 * t + 4          # key blocks for this q tile
                    osum = accp.tile([P, QTS], f32, tag="acc", name="osum")
                    rs = accp.tile([P, QTS], f32, tag="acc", name="rs")
                    npair = nj // 2

                    def rs_osum(p, pt):
                        for jj in range(2):
                            j = 2 * p + jj
                            co = max(0, P * j - qs)
                            nc.tensor.matmul(rs[:, co:QTS], ones[:],
                                             pt[:, jj, co:QTS],
                                             start=(j == 0), stop=(j == nj - 1))
                            nc.tensor.matmul(osum[:, co:QTS], Vn_sb[:, j, :],
                                             pt[:, jj, co:QTS],
                                             start=(j == 0), stop=(j == nj - 1))

                    pend = None
                    for pr in range(npair):
                        sp = sp2.tile([P, 2, QTS], f32, tag="sp")
                        co0 = max(0, P * 2 * pr - qs)
                        for jj in range(2):
                            j = 2 * pr + jj
                            co = max(0, P * j - qs)
                            diag = j >= 4 * t
                            nc.tensor.matmul(
                                sp[:, jj, co:QTS], KT_sb[:, ts(j, P)],
                                QT_sb[:, h, qs + co:qs + QTS],
                                start=True, stop=not diag)
                            if diag:
                                nc.tensor.matmul(sp[:, jj, co:co + P],
                                                 masknegT[:], ident[:],
                                                 start=False, stop=True)
                        pt = ptp.tile([P, 2, QTS], bf16, tag="pt")
                        nc.scalar.activation(pt[:, :, co0:QTS],
                                             sp[:, :, co0:QTS], Exp,
                                             scale=SCALE)
                        if pend is not None:
                            rs_osum(*pend)
                        pend = (pr, pt)
                    rs_osum(*pend)
                    # 1/rowsum via 2-op Newton-Raphson approx (~2 ULP) straight
                    # off PSUM (already partition-broadcast by the ones matmul).
                    rsc = nrm.tile([P, QTS], f32, tag="rsc")
                    recipB = nrm.tile([P, QTS], f32, tag="recipB")
                    nc.vector.reciprocal_approx_accurate(recipB[:, :], rs[:, :],
                                                         rsc[:, :])
                    nc.vector.tensor_mul(OT_sb[:, h, qs:qs + QTS], osum[:, :],
                                         recipB[:, :])
                # o_proj for the token blocks whose attention column is done
                for tb in range(4 * t, 4 * t + 4):
                    for ho in range(H // QTS):
                        yp = accp.tile([P, QTS], f32, tag="acc", name="yp")
                        for h in range(NH):
                            nc.tensor.matmul(yp[:, :], OT_sb[:, h, ts(tb, P)],
                                             woT_sb[:, h, ts(ho, QTS)],
                                             start=(h == 0), stop=(h == NH - 1))
                        yo = yop.tile([P, QTS], bf16, tag="yo")
                        nc.vector.tensor_copy(yo[:, :], yp[:, :])
                        nc.sync.dma_start(y[ts(tb, P), ts(ho, QTS)], yo[:, :])

    nc.compile()
    return nc


_NC_CACHE = None


def _get_nc():
    global _NC_CACHE
    if _NC_CACHE is None:
        _NC_CACHE = build_nc()
    return _NC_CACHE


def make_in_maps(hidden_states, position_ids, wq, wk, wv, wo):
    """Host-side sharding: 8 cores = (batch b = core//4) x (GQA group g = core%4)."""
    in_maps = []
    xTs, coss, srs = {}, {}, {}
    for b in range(2):
        xTs[b] = np.ascontiguousarray(hidden_states[b].T).astype(BF16)
        inv = 1.0 / (10000.0 ** (np.arange(0, P, 2, dtype=np.float64) / P))
        invd = np.concatenate([inv, inv]).astype(np.float64)
        fr = invd[:, None] * position_ids[b].astype(np.float64)[None, :]
        coss[b] = np.cos(fr).astype(BF16)
        sr = np.sin(fr).astype(np.float32)
        sr[:64] *= -1.0
        srs[b] = sr.astype(BF16)
    shards = {}
    for g in range(4):
        shards[g] = dict(
            wqT=np.ascontiguousarray(wq[DQ * g:DQ * (g + 1)].T).astype(BF16),
            wkT=np.ascontiguousarray(wk[P * g:P * (g + 1)].T).astype(BF16),
            wvT=np.ascontiguousarray(wv[P * g:P * (g + 1)].T).astype(BF16),
            woT=np.ascontiguousarray(wo[:, DQ * g:DQ * (g + 1)].T).astype(BF16),
        )
    for core in range(8):
        b, g = core // 4, core % 4
        in_maps.append(dict(xT=xTs[b], cosT=coss[b], sinrotT=srs[b], **shards[g]))
    return in_maps


def kernel(hidden_states, position_ids, wq, wk, wv, wo, **run_kwargs):
    nc = _get_nc()
    in_maps = make_in_maps(np.asarray(hidden_states), np.asarray(position_ids),
                           np.asarray(wq), np.asarray(wk), np.asarray(wv),
                           np.asarray(wo))
    res = run_bass_kernel_spmd(nc, in_maps, core_ids=list(range(8)), **run_kwargs)
    out = np.zeros((2, S, H), np.float32)
    for core in range(8):
        out[core // 4] += res.results[core]["y"].astype(np.float32)
    if run_kwargs:
        kernel.last_results = res
    return out



# revision 23
# speedup vs baseline: 1.3352x; 1.0080x over previous
"""GQA multi-head attention (B=2, S=2048, H=2048, 16 Q heads / 4 KV heads, RoPE,
causal) on 8 Trainium2 NeuronCores.

Sharding: tensor-parallel over GQA groups (4 groups, each 4 Q heads + 1 KV head)
x data-parallel over batch (2). Core c handles batch b = c // 4, group g = c % 4.
Column-parallel q/k/v projections, row-parallel o_proj; the 4 partial o_proj
outputs per batch are summed on the host.

Per-core kernel (all matmuls bf16 with fp32 PSUM accumulation):
  phase 1: Q^T/K^T/V^T projections from X^T, RoPE on Q/K (half-swap done with a
           PSUM->SBUF DMA partition swap), V transposed to natural layout on PE.
  phase 2: flash-style attention in S^T (keys x queries) layout: S^T = K^T.T Q^T
           per 128-key block, exp on ScalarE (no max subtraction - scores are
           bounded), causal mask via a 128x128 0/1 multiply on the diagonal
           blocks, key-axis row sums via a ones-vector matmul, O^T = V^T P^T
           accumulated in PSUM, normalized by 1/rowsum at the end.
  phase 3: row-parallel o_proj Y = O^T.T @ Wo_g^T accumulated over the 4 local
           heads, DMA'd straight from PSUM to HBM.
"""

import sys

for _p in ("/root/.axon_site", "/root/.axon_site/_ro/trn_rl_repo",
           "/root/.axon_site/_ro/pypackages", "/opt/trn_rl_repo"):
    if _p not in sys.path:
        sys.path.append(_p)

import numpy as np
import ml_dtypes

import concourse.bass as bass
import concourse.tile as tile
import concourse.mybir as mybir
from concourse import bacc
from concourse.bass import ts
from concourse.bass_utils import run_bass_kernel_spmd
from concourse.masks import make_identity, make_upper_triangular
from contextlib import ExitStack

BF16 = ml_dtypes.bfloat16
P = 128
S = 2048
H = 2048
NH = 4          # Q heads per core
DQ = NH * P     # 512
NCH = H // P    # 16 hidden chunks
NKB = S // P    # 16 key blocks
QTS = 512       # query tile (phase 2)
SCALE = 1.0 / float(np.sqrt(128.0))


def build_nc():
    f32 = mybir.dt.float32
    bf16 = mybir.dt.bfloat16
    nc = bacc.Bacc("TRN2", target_bir_lowering=False, debug=False)

    xT = nc.dram_tensor("xT", (H, S), bf16, kind="ExternalInput").ap()
    wqT = nc.dram_tensor("wqT", (H, DQ), bf16, kind="ExternalInput").ap()
    wkT = nc.dram_tensor("wkT", (H, P), bf16, kind="ExternalInput").ap()
    wvT = nc.dram_tensor("wvT", (H, P), bf16, kind="ExternalInput").ap()
    woT = nc.dram_tensor("woT", (DQ, H), bf16, kind="ExternalInput").ap()
    cosT = nc.dram_tensor("cosT", (P, S), bf16, kind="ExternalInput").ap()
    srT = nc.dram_tensor("sinrotT", (P, S), bf16, kind="ExternalInput").ap()
    y = nc.dram_tensor("y", (S, H), bf16, kind="ExternalOutput").ap()
    rss2 = nc.dram_tensor("rss2", (NH * S // QTS, QTS), f32).ap()  # recip scratch

    Exp = mybir.ActivationFunctionType.Exp

    with ExitStack() as ctx:
        tc = ctx.enter_context(tile.TileContext(nc))
        singles = ctx.enter_context(tc.tile_pool(name="singles", bufs=1))

        # Batched input loads: one strided DMA per group-of-chunks (per-DMA
        # issue on the Sync queue costs ~0.6us; 54 small loads would delay the
        # first matmul by ~30us). K/V weights first so the chunk-streamed K/V
        # projections start immediately.
        xT_sb = singles.tile([P, NCH, S], bf16)
        wqT_sb = singles.tile([P, NCH, DQ], bf16)
        wkT_sb = singles.tile([P, NCH, P], bf16)
        wvT_sb = singles.tile([P, NCH, P], bf16)
        xTr = xT.rearrange("(c p) s -> p c s", p=P)
        wqTr = wqT.rearrange("(c p) m -> p c m", p=P)
        cos_sb = singles.tile([P, S], bf16)
        sr_sb = singles.tile([P, S], bf16)
        nc.sync.dma_start(wkT_sb[:, :, :], wkT.rearrange("(c p) m -> p c m", p=P))
        for c in range(2):  # first chunks individually: K proj streams earliest
            nc.sync.dma_start(xT_sb[:, c, :], xTr[:, c, :])
        nc.sync.dma_start(cos_sb, cosT)
        nc.sync.dma_start(sr_sb, srT)
        nc.sync.dma_start(wvT_sb[:, :, :], wvT.rearrange("(c p) m -> p c m", p=P))
        for c in range(2, 8):
            nc.sync.dma_start(xT_sb[:, c, :], xTr[:, c, :])
        nc.sync.dma_start(wqT_sb[:, 0:8, :], wqTr[:, 0:8, :])
        for c in range(8, 12):
            nc.sync.dma_start(xT_sb[:, c, :], xTr[:, c, :])
        nc.sync.dma_start(wqT_sb[:, 8:16, :], wqTr[:, 8:16, :])
        for c in range(12, 16):
            nc.sync.dma_start(xT_sb[:, c, :], xTr[:, c, :])
        woT_sb = singles.tile([P, NH, H], bf16)
        nc.sync.dma_start(woT_sb[:, :, :], woT.rearrange("(c p) m -> p c m", p=P))

        # Additive causal mask, applied on the PE: a matmul of masknegT.T @ I
        # accumulated into the scores PSUM adds -1e9 where key > query.
        masknegT = singles.tile([P, P], bf16)
        make_upper_triangular(nc, masknegT[:], val=-1e9, diag=False)
        ident = singles.tile([P, P], bf16)
        make_identity(nc, ident[:])
        # Full [128,128] ones: the rowsum matmul then uses all PE column
        # groups (an M=1 matmul runs col_grp-restricted, and switching
        # col_grp between matmuls breaks back-to-back pipelining: 309ns
        # vs 216ns spacing), and every PSUM partition gets the rowsum,
        # which doubles as the partition-broadcast for the normalizer.
        ones = singles.tile([P, P], bf16)
        nc.vector.memset(ones[:], 1.0)

        QT_sb = singles.tile([P, NH, S], bf16)
        KT_sb = singles.tile([P, S], bf16)
        VT_sb = singles.tile([P, S], bf16)
        Vn_sb = singles.tile([P, NKB, P], bf16)
        OT_sb = singles.tile([P, NH, S], bf16)

        # ---------------- phase 1: projections + RoPE + V transpose ---------
        with tc.tile_pool(name="pj", bufs=4, space="PSUM") as pj, \
             tc.tile_pool(name="rope", bufs=2) as rp, \
             tc.tile_pool(name="vtp", bufs=2, space="PSUM") as vtp:

            # PE warmup during the initial DMA window: no-dep matmuls keep the
            # HAM activity monitor busy so real matmuls start at 2.4 GHz.
            # Issued before chunk-gated projection matmuls (strict PE FIFO), so
            # they fill each DMA wait instead of idling into a re-throttle.
            warm = vtp.tile([P, P], f32, tag="warm", bufs=1)

            def warmup(n):
                for _ in range(n):
                    nc.tensor.matmul(warm[:, :], ident[:], ident[:],
                                     start=True, stop=True)

            warmup(36)

            def proj(w_sb, head, tok):
                ps = pj.tile([P, QTS], f32, tag="proj")
                for c in range(NCH):
                    nc.tensor.matmul(
                        ps[:, :], w_sb[:, c, ts(head, P)], xT_sb[:, c, ts(tok, QTS)],
                        start=(c == 0), stop=(c == NCH - 1))
                return ps

            def proj_streamed(w_sb, fill=0):
                # chunk-outer: all 4 token tiles accumulate as xT chunks land,
                # so the PE starts ~1 chunk after the first DMA instead of
                # waiting for the full xT load.
                pss = [pj.tile([P, QTS], f32, tag="proj", name=f"pstr{t}")
                       for t in range(4)]
                for c in range(NCH):
                    if fill and c >= 2:
                        warmup(fill)
                    for t in range(4):
                        nc.tensor.matmul(
                            pss[t][:, :], w_sb[:, c, :], xT_sb[:, c, ts(t, QTS)],
                            start=(c == 0), stop=(c == NCH - 1))
                return pss

            def rope(ps, out_region, tok):
                qf = rp.tile([P, QTS], f32, tag="qf")
                nc.scalar.copy(qf[:, :], ps[:, :])
                sw = rp.tile([P, QTS], f32, tag="swap")
                # SWDGE (gpsimd) queue: the sync queue is busy issuing the
                # input loads for the first ~40us and would delay K's rope.
                nc.gpsimd.dma_start(sw[0:64, :], qf[64:128, :])
                nc.gpsimd.dma_start(sw[64:128, :], qf[0:64, :])
                t1 = rp.tile([P, QTS], f32, tag="t1")
                nc.vector.tensor_mul(t1[:, :], ps[:, :], cos_sb[:, ts(tok, QTS)])
                t2 = rp.tile([P, QTS], f32, tag="t2")
                nc.vector.tensor_mul(t2[:, :], sw[:, :], sr_sb[:, ts(tok, QTS)])
                nc.vector.tensor_add(out_region, t1[:, :], t2[:, :])

            # K and V first (chunk-streamed): phase 2 consumes them first.
            for t, ps in enumerate(proj_streamed(wkT_sb, fill=8)):
                rope(ps, KT_sb[:, ts(t, QTS)], t)
            for t, ps in enumerate(proj_streamed(wvT_sb)):
                nc.vector.tensor_copy(VT_sb[:, ts(t, QTS)], ps[:, :])
                for b in range(4 * t, 4 * t + 4):
                    tp = vtp.tile([P, P], bf16, tag="vt")
                    nc.tensor.transpose(tp[:, :], VT_sb[:, ts(b, P)], ident[:])
                    nc.vector.tensor_copy(Vn_sb[:, b, :], tp[:, :])
            # t-outer so attention column t=0 (all heads) unblocks first
            for t in range(S // QTS):
                for h in range(NH):
                    ps = proj(wqT_sb, h, t)
                    rope(ps, QT_sb[:, h, ts(t, QTS)], t)

        # ------- phase 2 + 3: attention with o_proj interleaved per q column -
        # Key blocks are processed in PAIRS sharing one [128, 2, 512] PSUM
        # scores tile so each exp ACTIVATE covers ~1024 elems/partition: the
        # 352-cycle fixed overhead per ACTIVATE made per-block exp (687ns)
        # slower than the 642ns of PE work per block - attention was
        # scalar-bound.  Paired, exp is ~1147ns vs ~1284ns PE work per pair.
        # rs/osum for pair p-1 are emitted after scores+exp of pair p, so each
        # exp hides under already-ready PE work (PE queue is strict FIFO).
        # osum / rowsum / o_proj accumulators share one 4-buffer PSUM ring
        # ("acc") to fit: 2*2 (sp) + 4 (acc) = 8 banks.
        with tc.tile_pool(name="sp2", bufs=2, space="PSUM") as sp2, \
             tc.tile_pool(name="accp", bufs=4, space="PSUM") as accp, \
             tc.tile_pool(name="ptp", bufs=3) as ptp, \
             tc.tile_pool(name="yop", bufs=4) as yop, \
             tc.tile_pool(name="nrm", bufs=3) as nrm:
            for t in range(S // QTS):
                for h in range(NH):
                    qs = QTS * t
                    nj = 4 * t + 4          # key blocks for this q tile
                    osum = accp.tile([P, QTS], f32, tag="acc", name="osum")
                    rs = accp.tile([P, QTS], f32, tag="acc", name="rs")
                    npair = nj // 2

                    def rs_osum(p, pt):
                        for jj in range(2):
                            j = 2 * p + jj
                            co = max(0, P * j - qs)
                            nc.tensor.matmul(rs[:, co:QTS], ones[:],
                                             pt[:, jj, co:QTS],
                                             start=(j == 0), stop=(j == nj - 1))
                            nc.tensor.matmul(osum[:, co:QTS], Vn_sb[:, j, :],
                                             pt[:, jj, co:QTS],
                                             start=(j == 0), stop=(j == nj - 1))

                    pend = None
                    for pr in range(npair):
                        sp = sp2.tile([P, 2, QTS], f32, tag="sp")
                        co0 = max(0, P * 2 * pr - qs)
                        for jj in range(2):
                            j = 2 * pr + jj
                            co = max(0, P * j - qs)
                            diag = j >= 4 * t
                            nc.tensor.matmul(
                                sp[:, jj, co:QTS], KT_sb[:, ts(j, P)],
                                QT_sb[:, h, qs + co:qs + QTS],
                                start=True, stop=not diag)
                            if diag:
                                nc.tensor.matmul(sp[:, jj, co:co + P],
                                                 masknegT[:], ident[:],
                                                 start=False, stop=True)
                        pt = ptp.tile([P, 2, QTS], bf16, tag="pt")
                        nc.scalar.activation(pt[:, :, co0:QTS],
                                             sp[:, :, co0:QTS], Exp,
                                             scale=SCALE)
                        if pend is not None:
                            rs_osum(*pend)
                        pend = (pr, pt)
                    rs_osum(*pend)
                    # 1/rowsum via 2-op Newton-Raphson approx (~2 ULP) straight
                    # off PSUM (already partition-broadcast by the ones matmul).
                    rsc = nrm.tile([P, QTS], f32, tag="rsc")
                    recipB = nrm.tile([P, QTS], f32, tag="recipB")
                    nc.vector.reciprocal_approx_accurate(recipB[:, :], rs[:, :],
                                                         rsc[:, :])
                    nc.vector.tensor_mul(OT_sb[:, h, qs:qs + QTS], osum[:, :],
                                         recipB[:, :])
                # o_proj for the token blocks whose attention column is done
                for tb in range(4 * t, 4 * t + 4):
                    for ho in range(H // QTS):
                        yp = accp.tile([P, QTS], f32, tag="acc", name="yp")
                        for h in range(NH):
                            nc.tensor.matmul(yp[:, :], OT_sb[:, h, ts(tb, P)],
                                             woT_sb[:, h, ts(ho, QTS)],
                                             start=(h == 0), stop=(h == NH - 1))
                        yo = yop.tile([P, QTS], bf16, tag="yo")
                        nc.vector.tensor_copy(yo[:, :], yp[:, :])
                        nc.sync.dma_start(y[ts(tb, P), ts(ho, QTS)], yo[:, :])

    nc.compile()
    return nc


_NC_CACHE = None


def _get_nc():
    global _NC_CACHE
    if _NC_CACHE is None:
        _NC_CACHE = build_nc()
    return _NC_CACHE


def make_in_maps(hidden_states, position_ids, wq, wk, wv, wo):
    """Host-side sharding: 8 cores = (batch b = core//4) x (GQA group g = core%4)."""
    in_maps = []
    xTs, coss, srs = {}, {}, {}
    for b in range(2):
        xTs[b] = np.ascontiguousarray(hidden_states[b].T).astype(BF16)
        inv = 1.0 / (10000.0 ** (np.arange(0, P, 2, dtype=np.float64) / P))
        invd = np.concatenate([inv, inv]).astype(np.float64)
        fr = invd[:, None] * position_ids[b].astype(np.float64)[None, :]
        coss[b] = np.cos(fr).astype(BF16)
        sr = np.sin(fr).astype(np.float32)
        sr[:64] *= -1.0
        srs[b] = sr.astype(BF16)
    shards = {}
    for g in range(4):
        shards[g] = dict(
            wqT=np.ascontiguousarray(wq[DQ * g:DQ * (g + 1)].T).astype(BF16),
            wkT=np.ascontiguousarray(wk[P * g:P * (g + 1)].T).astype(BF16),
            wvT=np.ascontiguousarray(wv[P * g:P * (g + 1)].T).astype(BF16),
            woT=np.ascontiguousarray(wo[:, DQ * g:DQ * (g + 1)].T).astype(BF16),
        )
    for core in range(8):
        b, g = core // 4, core % 4
        in_maps.append(dict(xT=xTs[b], cosT=coss[b], sinrotT=srs[b], **shards[g]))
    return in_maps


def kernel(hidden_states, position_ids, wq, wk, wv, wo, **run_kwargs):
    nc = _get_nc()
    in_maps = make_in_maps(np.asarray(hidden_states), np.asarray(position_ids),
                           np.asarray(wq), np.asarray(wk), np.asarray(wv),
                           np.asarray(wo))
    res = run_bass_kernel_spmd(nc, in_maps, core_ids=list(range(8)), **run_kwargs)
    out = np.zeros((2, S, H), np.float32)
    for core in range(8):
        out[core // 4] += res.results[core]["y"].astype(np.float32)
    if run_kwargs:
        kernel.last_results = res
    return out

